# revision 3
# baseline (speedup 1.0000x reference)
"""AttentiveConv3d (sparse_attention) Trainium2 kernel — self-contained.

kernel(**inputs) takes the FULL inputs
    x     [2, 128, 16, 28, 28] f32
    q     [2, 1, 64] f32
    W_out [128, 128] f32
    b_out [128] f32
and returns the FULL output [2, 128, 16, 28, 28] f32.

Sharding: data-parallel over (batch, T-chunks): 8 cores, core i handles
batch i//4, output frames 4*(i%4) .. 4*(i%4)+3, with a 1-frame halo supplied
by host-side padding/slicing (no device collectives needed).

Math (equivalent to the reference; softmax computed without max-subtraction,
valid because |logits| < ~0.2 for this operator's scaling):
    z   = qmask^T @ xp        (per padded location; both heads)
    E   = exp(z);  F = E * xp
    num = Box3x3x3(F); d = Box3x3x3(E)    (separable box filters)
    y   = W_out @ (num / d) + b_out
"""
from contextlib import ExitStack

import numpy as np

import concourse.bass as bass
import concourse.tile as tile
from concourse import bacc, mybir
from concourse import bass_utils

F32 = mybir.dt.float32
F32R = mybir.dt.float32r
AF = mybir.ActivationFunctionType

C = 128
TIN, TOUT = 6, 4
HP, WP = 30, 30
HO, WO = 28, 28
NF = HP * WP
NOF = HO * WO


def _build_nc(num_devices=8, f_on_pool=True, reps=1):
    nc = bacc.Bacc("TRN2", target_bir_lowering=False, debug=False,
                   num_devices=num_devices)
    d_xp = nc.dram_tensor("xp", [C, TIN, HP, WP], F32R, kind="ExternalInput").ap()
    d_qm = nc.dram_tensor("qm", [C, C], F32R, kind="ExternalInput").ap()
    d_id = nc.dram_tensor("idm", [C, C], F32R, kind="ExternalInput").ap()
    d_sel = nc.dram_tensor("sel", [8, TOUT, C], F32R, kind="ExternalInput").ap()
    d_wt = nc.dram_tensor("wt", [C, C], F32R, kind="ExternalInput").ap()
    d_bias = nc.dram_tensor("bias", [C, 1], F32, kind="ExternalInput").ap()
    d_y = nc.dram_tensor("y", [C, TOUT, HO, WO], F32, kind="ExternalOutput").ap()

    with tile.TileContext(nc) as tc:
        with ExitStack() as ctx:
            consts = ctx.enter_context(tc.tile_pool(name="consts", bufs=1))
            sb_x = ctx.enter_context(tc.tile_pool(name="sb_x", bufs=1))
            sb_e = ctx.enter_context(tc.tile_pool(name="sb_e", bufs=1))
            sb_f = ctx.enter_context(tc.tile_pool(name="sb_f", bufs=1))
            sb_g = ctx.enter_context(tc.tile_pool(name="sb_g", bufs=2))
            sb_tmp = ctx.enter_context(tc.tile_pool(name="sb_tmp", bufs=2))
            sb_m = ctx.enter_context(tc.tile_pool(name="sb_m", bufs=2))
            sb_y = ctx.enter_context(tc.tile_pool(name="sb_y", bufs=2))
            sb_ep = ctx.enter_context(tc.tile_pool(name="sb_ep", bufs=1))
            ps_big = ctx.enter_context(tc.tile_pool(name="ps_big", bufs=2, space="PSUM"))
            ps_small = ctx.enter_context(tc.tile_pool(name="ps_small", bufs=4, space="PSUM"))

            qm_t = consts.tile([C, C], F32R)
            nc.sync.dma_start(out=qm_t[:], in_=d_qm[:])
            id_t = consts.tile([C, C], F32R)
            nc.sync.dma_start(out=id_t[:], in_=d_id[:])
            sel_t = consts.tile([8, TOUT, C], F32R)
            nc.sync.dma_start(out=sel_t[:], in_=d_sel[:])
            wt_t = consts.tile([C, C], F32R)
            nc.sync.dma_start(out=wt_t[:], in_=d_wt[:])
            bias_t = consts.tile([C, 1], F32)
            nc.sync.dma_start(out=bias_t[:], in_=d_bias[:])

            for _ in range(reps):
                _body(nc, d_xp, d_y, qm_t, id_t, sel_t, wt_t, bias_t,
                      sb_x, sb_e, sb_f, sb_g, sb_tmp, sb_m, sb_y, sb_ep,
                      ps_big, ps_small, f_on_pool)
    nc.compile()
    return nc


def _body(nc, d_xp, d_y, qm_t, id_t, sel_t, wt_t, bias_t,
          sb_x, sb_e, sb_f, sb_g, sb_tmp, sb_m, sb_y, sb_ep,
          ps_big, ps_small, f_on_pool):
    # ---- phase A: load, z, exp, F ------------------------------------
    e128 = sb_e.tile([C, TIN * NF], F32R, tag="e128")
    f_tiles = []
    for f in range(TIN):
        xt = sb_x.tile([C, NF], F32R, tag=f"x{f}", name=f"xt{f}")
        nc.sync.dma_start(out=xt[:], in_=d_xp[:, f])

        zp = ps_big.tile([C, 1024], F32, tag="big", name=f"zp{f}")
        nc.tensor.matmul(zp[:, 0:450], qm_t[:], xt[:, 0:450], start=True, stop=True)
        nc.tensor.matmul(zp[:, 512:962], qm_t[:], xt[:, 450:900], start=True, stop=True)

        # E128 free layout is y-major: (y, t, x)
        zv = zp[:].rearrange("p (b k) -> p b k", b=2)[:, :, 0:450]
        e128v = e128[:].rearrange("p (y t x) -> p y t x", y=HP, t=TIN)
        ev = (e128v[:, :, f:f + 1, :]
              .rearrange("p (c y) o x -> p c y (o x)", c=2))
        nc.scalar.activation(ev, zv, AF.Exp)

        ft = sb_f.tile([C, NF], F32R, tag=f"f{f}", name=f"ft{f}")
        e_slice = e128v[:, :, f:f + 1, :]
        if f_on_pool:
            nc.gpsimd.tensor_mul(ft[:], e_slice, xt[:])
        else:
            nc.vector.tensor_mul(ft[:], e_slice, xt[:])
        f_tiles.append(ft)

    # ---- phase B: denominator path -----------------------------------
    ep1 = sb_ep.tile([60, TIN, WP], F32R, tag="ep1")
    for h in range(2):
        nc.scalar.dma_start(
            out=ep1[30 * h:30 * h + 30, :, :].rearrange("p t x -> p (t x)"),
            in_=e128[h:h + 1, :].rearrange("p (y q) -> p y q", y=HP))
    ew = sb_ep.tile([60, TIN, WO], F32, tag="ew")
    et1 = sb_ep.tile([60, TIN, WO], F32, tag="et1")
    nc.vector.tensor_add(et1[:], ep1[:, :, 0:28].bitcast(F32), ep1[:, :, 1:29].bitcast(F32))
    nc.vector.tensor_add(ew[:], et1[:], ep1[:, :, 2:30].bitcast(F32))
    ewt = sb_ep.tile([60, TOUT, WO], F32, tag="ewt")
    et2 = sb_ep.tile([60, TOUT, WO], F32, tag="et2")
    nc.vector.tensor_add(et2[:], ew[:, 0:4, :], ew[:, 1:5, :])
    nc.vector.tensor_add(ewt[:], et2[:], ew[:, 2:6, :])
    ep2 = sb_ep.tile([8, HP, WO], F32, tag="ep2")
    for h in range(2):
        for t in range(TOUT):
            eng = nc.sync if t % 2 == 0 else nc.scalar
            eng.dma_start(out=ep2[4 * h + t:4 * h + t + 1, :, :],
                          in_=ewt[30 * h:30 * h + 30, t:t + 1, :])
    d8 = sb_ep.tile([8, HO, WO], F32, tag="d8")
    et3 = sb_ep.tile([8, HO, WO], F32, tag="et3")
    nc.vector.tensor_add(et3[:], ep2[:, 0:28, :], ep2[:, 1:29, :])
    nc.vector.tensor_add(d8[:], et3[:], ep2[:, 2:30, :])
    r8f = sb_ep.tile([8, HO * WO], F32, tag="r8f")
    nc.vector.reciprocal_approx_fast(r8f[:], d8[:].rearrange("p y x -> p (y x)"))
    r8 = sb_ep.tile([8, HO * WO], F32R, tag="r8")
    nc.vector.tensor_copy(r8[:], r8f[:])

    # ---- phase C: per output frame -----------------------------------
    for t in range(TOUT):
        ftp = ps_big.tile([C, 1024], F32, tag="big", name=f"ftp{t}")
        for half in range(2):
            lo, hi = half * 512, half * 512 + 450
            slo = half * 450
            for dt in range(3):
                nc.tensor.matmul(ftp[:, lo:hi], id_t[:],
                                 f_tiles[t + dt][:, slo:slo + 450],
                                 start=(dt == 0), stop=(dt == 2))
        ftv = (ftp[:].rearrange("p (b k) -> p b k", b=2)[:, :, 0:450]
               .rearrange("p b (r x) -> p b r x", x=WP))
        # W-pass: DVE may read only ONE psum operand per op, so the middle
        # tap goes through an ACT copy to SBUF first.
        cpw = sb_tmp.tile([C, HP, WO], F32, tag="cpw", name=f"cpw{t}")
        cpv = cpw[:].rearrange("p (b r) x -> p b r x", b=2)
        nc.scalar.copy(cpv, ftv[:, :, :, 1:29])
        gt = sb_g.tile([C, HP, WO], F32R, tag="g", name=f"gt{t}")
        gv = gt[:].rearrange("p (b r) x -> p b r x", b=2)
        wtmp = sb_tmp.tile([C, HP, WO], F32, tag="wtmp", name=f"wtmp{t}")
        wv = wtmp[:].rearrange("p (b r) x -> p b r x", b=2)
        nc.vector.tensor_add(wv, cpv, ftv[:, :, :, 0:28])
        nc.vector.tensor_add(gv, wv, ftv[:, :, :, 2:30])

        rsbs = []
        for ch in range(2):
            rp = ps_small.tile([C, 392], F32, tag="small", name=f"rp{t}_{ch}")
            nc.tensor.matmul(rp[:], sel_t[:, t, :], r8[:, ch * 392:ch * 392 + 392],
                             start=True, stop=True)
            rsb = sb_tmp.tile([C, 392], F32, tag="rsb", name=f"rsb{t}_{ch}", bufs=4)
            nc.scalar.copy(rsb[:], rp[:])
            rsbs.append(rsb)

        mt = sb_m.tile([C, NOF], F32R, tag="m", name=f"mt{t}")
        for ch in range(2):
            nump = ps_small.tile([C, 392], F32, tag="small", name=f"nump{t}_{ch}")
            for dy in range(3):
                nc.tensor.matmul(nump[:], id_t[:],
                                 gt[:, dy + 14 * ch: dy + 14 * ch + 14, :],
                                 start=(dy == 0), stop=(dy == 2))
            nc.vector.tensor_mul(mt[:, ch * 392:ch * 392 + 392], nump[:], rsbs[ch][:])

        yt = sb_y.tile([C, NOF], F32, tag="y", name=f"yt{t}")
        for ch in range(2):
            yp = ps_small.tile([C, 392], F32, tag="small", name=f"yp{t}_{ch}")
            nc.tensor.matmul(yp[:], wt_t[:], mt[:, ch * 392:ch * 392 + 392],
                             start=True, stop=True)
            nc.scalar.activation(yt[:, ch * 392:ch * 392 + 392], yp[:],
                                 AF.Identity, bias=bias_t[:], scale=1.0)
        nc.scalar.dma_start(out=d_y[:, t], in_=yt[:])


# ---------------------------------------------------------------------------
# Host side
# ---------------------------------------------------------------------------

def _host_prep(x, q, W_out, b_out):
    B, C_, T, H, W = x.shape
    heads, hs = 2, 64
    xpad = np.zeros((B, C_, T + 2, HP, WP), np.float32)
    xpad[:, :, 1:T + 1, 1:H + 1, 1:W + 1] = x

    cidx = np.arange(C_)
    qfull = (np.asarray(q, np.float32)[cidx % heads, 0, cidx // heads] / hs)
    qm = np.zeros((C_, C_), np.float32)
    for m in range(C_):
        qm[:, m] = np.where(cidx % heads == m % heads, qfull, 0.0)
    idm = np.eye(C_, dtype=np.float32)
    sel = np.zeros((8, TOUT, C_), np.float32)
    for t in range(TOUT):
        sel[4 * (cidx % heads) + t, t, cidx] = 1.0
    wt = np.ascontiguousarray(np.asarray(W_out, np.float32).T)
    bias = np.asarray(b_out, np.float32).reshape(C_, 1)

    shared = {"qm": qm, "idm": idm, "sel": sel, "wt": wt, "bias": bias}
    in_maps = []
    for core in range(8):
        b, t0 = core // 4, (core % 4) * 4
        xp = np.ascontiguousarray(xpad[b, :, t0:t0 + TIN])
        in_maps.append({"xp": xp, **shared})
    return in_maps


_NC_CACHE = {}


def _get_nc(reps=1):
    if reps not in _NC_CACHE:
        _NC_CACHE[reps] = _build_nc(reps=reps)
    return _NC_CACHE[reps]


def kernel(x, q, W_out, b_out):
    x = np.asarray(x, np.float32)
    in_maps = _host_prep(x, q, W_out, b_out)
    nc = _get_nc()
    res = bass_utils.run_bass_kernel_spmd(nc, in_maps, list(range(8)))
    y = np.zeros((2, 128, 16, 28, 28), np.float32)
    for core in range(8):
        b, t0 = core // 4, (core % 4) * 4
        y[b, :, t0:t0 + TOUT] = res.results[core]["y"]
    return y
